# revision 1
# baseline (speedup 1.0000x reference)
"""Trainium2 Bass kernel for nn_DistanceNorm.

Computation (B=64, L=2048, M=256), per batch b:
    px    = x[b].sum(axis=0); px /= px.sum()          (density over M bins)
    mean  = sum(px * rng);  std = sqrt(sum(px*(rng-mean)^2))   rng = arange(M)-127
    u[m]  = clip(rng[m]*std/25.6 + mean + 127, -1, 256)
    out[b,l,m] = lerp of x[b,l,:] at position u[m] (zero outside [0,255])

Key identity: the gather+lerp along M is a matmul with the triangle-kernel
matrix  G[r,m] = relu(1 - |r - u[m]|):   out[b] = x[b] @ G[b].

Per-core program (8 batches per core, batch dim sharded over 8 cores):
  - load x[b] (contiguous DMA)
  - PE transposes 128x128 tiles (4 per PSUM bank) -> xT (M on partitions);
    the packed PSUM->SBUF copies (alternating DVE/ACT) also emit row sums
    (accum_out) -> px tilde
  - tiny stats matmuls + scalar chain -> u row; ones-matmul broadcasts u
    across partitions; two DVE ops build -G = min(|iota-u|,1) - 1
  - 2 accumulating matmuls per 128-row output chunk apply G (negated; the
    PSUM->SBUF copy multiplies by -1)
  - contiguous DMA out
"""

from contextlib import ExitStack

import numpy as np

import concourse.bass as bass
import concourse.tile as tile
from concourse import bacc, mybir
from concourse.bass_utils import run_bass_kernel_spmd

B, L, M = 64, 2048, 256
N_CORES = 8
BPC = B // N_CORES          # batches per core
LCH = L // 128              # 16 l-chunks per batch
RCH = M // 128              # 2 r-chunks (contraction over M)
TPACK = 4                   # transposes packed per PSUM bank

F32 = mybir.dt.float32
F32R = mybir.dt.float32r

# main gather matmul dtype: float32r streams 4x faster than float32 on the
# PE; flip to False if hardware float32r precision is insufficient.
MAIN_F32R = False


def _consts():
    rng = np.arange(M, dtype=np.float64) - (M // 2) + 1.0          # -127..128
    denom = np.float64(np.float32(M) * np.float32(0.1))            # 25.6 as f32
    rng_over = (rng / denom).astype(np.float32).reshape(1, M)      # rng/25.6
    rmat = np.stack([np.ones(M), rng, rng * rng], axis=1).astype(np.float32)
    rmat = rmat.reshape(RCH, 128, 3)                               # [rc, r, k]
    iota = np.arange(128, dtype=np.float32)
    iota_cols = np.stack([iota + 128.0 * rc for rc in range(RCH)], axis=1)
    ident = np.eye(128, dtype=np.float32)
    ones_row = np.ones((1, 128), dtype=np.float32)
    return rng_over, rmat, iota_cols, ident, ones_row


def build_program(main_f32r=MAIN_F32R, reps=1):
    nc = bacc.Bacc("TRN2", target_bir_lowering=False, debug=False)

    x_dram = nc.dram_tensor("distance", [BPC, L, M], F32, kind="ExternalInput")
    out_dram = nc.dram_tensor("out", [BPC, L, M], F32, kind="ExternalOutput")

    rng_over, rmat, iota_cols, ident, ones_row = _consts()
    rng_dram = nc.inline_tensor(rng_over, "c_rng")
    rmat_dram = nc.inline_tensor(rmat, "c_rmat")
    iota_dram = nc.inline_tensor(iota_cols, "c_iota")
    ident_dram = nc.inline_tensor(ident, "c_ident")
    ones_dram = nc.inline_tensor(ones_row, "c_ones")

    mdt = F32R if main_f32r else F32

    with tile.TileContext(nc) as tc, ExitStack() as ctx:
        cpool = ctx.enter_context(tc.tile_pool(name="consts", bufs=1))
        xin_pool = ctx.enter_context(tc.tile_pool(name="xin", bufs=3))
        xt_pool = ctx.enter_context(tc.tile_pool(name="xt", bufs=2 * RCH))
        g_pool = ctx.enter_context(tc.tile_pool(name="g", bufs=2 * RCH))
        osb_pool = ctx.enter_context(tc.tile_pool(name="osb", bufs=3))
        st_pool = ctx.enter_context(tc.tile_pool(name="stats", bufs=2))
        ps_t = ctx.enter_context(tc.tile_pool(name="ps_t", bufs=4, space="PSUM"))
        ps_o = ctx.enter_context(tc.tile_pool(name="ps_o", bufs=2, space="PSUM"))
        ps_u = ctx.enter_context(tc.tile_pool(name="ps_u", bufs=1, space="PSUM"))
        ps_s = ctx.enter_context(tc.tile_pool(name="ps_s", bufs=1, space="PSUM"))

        c_rng = cpool.tile([1, M], F32, tag="c_rng")
        nc.sync.dma_start(c_rng[:], rng_dram.ap())
        c_rmat = cpool.tile([128, RCH, 3], F32, tag="c_rmat")
        nc.sync.dma_start(c_rmat[:], rmat_dram.ap().rearrange("rc r k -> r rc k"))
        c_iota = cpool.tile([128, RCH], F32, tag="c_iota")
        nc.sync.dma_start(c_iota[:], iota_dram.ap())
        c_ident = cpool.tile([128, 128], F32, tag="c_ident")
        nc.sync.dma_start(c_ident[:], ident_dram.ap())
        c_ones = cpool.tile([1, 128], F32, tag="c_ones")
        nc.sync.dma_start(c_ones[:], ones_dram.ap())

        copy_flip = 0  # round-robin PSUM->SBUF copies across DVE and ACT

        for b in [b for _ in range(reps) for b in range(BPC)]:
            # ---- load x[b]: (2048, 256) -> sbuf (128, lc=16, 256)
            xin = xin_pool.tile([128, LCH, M], F32, tag="xin")
            xr = x_dram.ap()[b].rearrange("(lc p) m -> p lc m", p=128)
            nc.sync.dma_start(xin[:], xr[:])

            # ---- transpose to xT[rc] (128, 2048); px-partials via accum_out
            xt = [xt_pool.tile([128, L], mdt, tag="xt", name=f"xt{rc}")
                  for rc in range(RCH)]
            ngrp = LCH // TPACK
            acc = st_pool.tile([128, RCH, ngrp], F32, tag="acc")
            for j in range(ngrp):
                for rc in range(RCH):
                    tp = ps_t.tile([128, TPACK * 128], F32, tag="tp")
                    for i in range(TPACK):
                        lc = TPACK * j + i
                        nc.tensor.transpose(
                            tp[:, 128 * i : 128 * (i + 1)],
                            xin[:, lc, 128 * rc : 128 * (rc + 1)],
                            c_ident[:],
                        )
                    dst = xt[rc][:, TPACK * 128 * j : TPACK * 128 * (j + 1)]
                    if copy_flip % 2 == 0:
                        nc.vector.tensor_scalar(
                            out=dst,
                            in0=tp[:],
                            scalar1=0.0,
                            scalar2=None,
                            op0=mybir.AluOpType.add,
                            op1=mybir.AluOpType.add,
                            accum_out=acc[:, rc, j : j + 1],
                        )
                    else:
                        nc.scalar.activation(
                            dst,
                            tp[:],
                            mybir.ActivationFunctionType.Copy,
                            accum_out=acc[:, rc, j : j + 1],
                        )
                    copy_flip += 1

            # ---- stats: [S, T1, T2] = sum_r pxt[r] * [1, rng, rng^2]
            pxt = st_pool.tile([128, RCH], F32, tag="pxt")
            for rc in range(RCH):
                nc.vector.tensor_reduce(
                    out=pxt[:, rc : rc + 1],
                    in_=acc[:, rc, :],
                    axis=mybir.AxisListType.X,
                    op=mybir.AluOpType.add,
                )
            ps_stats = ps_s.tile([1, 3], F32, tag="ps_stats")
            for rc in range(RCH):
                nc.tensor.matmul(
                    ps_stats[:],
                    pxt[:, rc : rc + 1],
                    c_rmat[:, rc, :],
                    start=(rc == 0),
                    stop=(rc == RCH - 1),
                )
            st = st_pool.tile([1, 8], F32, tag="st")
            # st layout: 0:S 1:T1 2:T2 3:recipS 4:mean 5:m2 6:var 7:std
            nc.vector.tensor_scalar(
                out=st[:, 0:3], in0=ps_stats[:], scalar1=0.0, scalar2=None,
                op0=mybir.AluOpType.add,
            )
            nc.vector.reciprocal(st[:, 3:4], st[:, 0:1])
            nc.vector.tensor_mul(st[:, 4:5], st[:, 1:2], st[:, 3:4])
            nc.vector.tensor_mul(st[:, 5:6], st[:, 2:3], st[:, 3:4])
            # var = m2 - mean^2
            nc.vector.tensor_tensor(
                out=st[:, 6:7], in0=st[:, 4:5], in1=st[:, 4:5],
                op=mybir.AluOpType.mult,
            )
            nc.vector.tensor_sub(st[:, 6:7], st[:, 5:6], st[:, 6:7])
            nc.scalar.sqrt(st[:, 7:8], st[:, 6:7])
            meanp = st_pool.tile([1, 1], F32, tag="meanp")
            nc.vector.tensor_scalar_add(meanp[:], st[:, 4:5], float(M // 2 - 1))

            # u = clip(rng/25.6 * std + (mean + 127), -1, 256)
            u_row = st_pool.tile([1, M], F32, tag="u_row")
            nc.vector.tensor_scalar(
                out=u_row[:], in0=c_rng[:],
                scalar1=st[:, 7:8], scalar2=meanp[:],
                op0=mybir.AluOpType.mult, op1=mybir.AluOpType.add,
            )
            nc.vector.tensor_scalar(
                out=u_row[:], in0=u_row[:],
                scalar1=-1.0, scalar2=float(M),
                op0=mybir.AluOpType.max, op1=mybir.AluOpType.min,
            )

            # ---- broadcast u across partitions; build -G = min(|u-iota|,1)-1
            ps_ub = ps_u.tile([128, M], F32, tag="ps_ub")
            nc.tensor.matmul(ps_ub[:], c_ones[:], u_row[:], start=True, stop=True)
            g = [g_pool.tile([128, M], mdt, tag="g", name=f"g{rc}")
                 for rc in range(RCH)]
            for rc in range(RCH):
                d = g_pool.tile([128, M], F32, tag="absd")
                nc.scalar.activation(
                    d[:], ps_ub[:], mybir.ActivationFunctionType.Abs,
                    bias=c_iota[:, rc : rc + 1], scale=-1.0,
                )
                nc.vector.tensor_scalar(
                    out=g[rc][:], in0=d[:],
                    scalar1=1.0, scalar2=1.0,
                    op0=mybir.AluOpType.min, op1=mybir.AluOpType.subtract,
                )

            # ---- main: -out[lc] = sum_rc xt[rc][:,lc-chunk].T @ (-g[rc])
            osb = osb_pool.tile([128, LCH, M], F32, tag="osb")
            for lc in range(LCH):
                po = ps_o.tile([128, M], F32, tag="po")
                for rc in range(RCH):
                    nc.tensor.matmul(
                        po[:],
                        xt[rc][:, 128 * lc : 128 * (lc + 1)],
                        g[rc][:],
                        start=(rc == 0),
                        stop=(rc == RCH - 1),
                    )
                if copy_flip % 2 == 0:
                    nc.vector.tensor_scalar(
                        out=osb[:, lc, :], in0=po[:],
                        scalar1=-1.0, scalar2=None, op0=mybir.AluOpType.mult,
                    )
                else:
                    nc.scalar.activation(
                        osb[:, lc, :], po[:],
                        mybir.ActivationFunctionType.Copy, scale=-1.0,
                    )
                copy_flip += 1

            orr = out_dram.ap()[b].rearrange("(lc p) m -> p lc m", p=128)
            nc.sync.dma_start(orr[:], osb[:])

    nc.compile()
    return nc


_NC_CACHE = None


def _get_program():
    global _NC_CACHE
    if _NC_CACHE is None:
        _NC_CACHE = build_program()
    return _NC_CACHE


def kernel(distance: np.ndarray) -> np.ndarray:
    assert distance.shape == (B, L, M), distance.shape
    x = np.ascontiguousarray(distance, dtype=np.float32)
    nc = _get_program()
    in_maps = [{"distance": x[i * BPC : (i + 1) * BPC]} for i in range(N_CORES)]
    res = run_bass_kernel_spmd(nc, in_maps, core_ids=list(range(N_CORES)))
    return np.concatenate([res.results[i]["out"] for i in range(N_CORES)], axis=0)

